# revision 1
# baseline (speedup 1.0000x reference)
"""SpecAugment (log-mel masking) Trainium2 kernel.

Full inputs: x [64,128,3000] f32, f0/f_w/t0/t_w [64,2] i32.
out[b,f,t] = fill_b if (f in freq band) or (t in time band) else x[b,f,t],
fill_b = min over x[b].

Strategy: batch-shard B=64 across 8 cores (8 samples/core). The int mask
params are tiny host tensors, so the per-sample 0/1 mask vectors are
computed on host and shipped as bf16 data; the device does only the
memory-bound work. Per sample:
  - DMA x[b] [128,3000] -> SBUF
  - DVE reduce_min (free axis) -> [128,1]; tiny DMA gather -> [1,128];
    reduce_min -> fill [1,1]; broadcast to [128,1] via tiny PE matmul
  - combined mask = ones(x)mt + mf(x)ones as ONE K=2 bf16 matmul per
    512-col chunk into PSUM (values {0,1,2}; nonzero == masked)
  - DVE copy_predicated overwrites masked cells with fill (data operand
    is fill128 broadcast along the free axis)
  - DMA xt -> y[b]
HBM traffic is the minimum 2 x 12.3MB per core -> ~69us roofline.
"""

import ml_dtypes
import numpy as np

import concourse.bacc as bacc
import concourse.bass as bass
import concourse.mybir as mybir
import concourse.tile as tile
import concourse.bass_utils as bass_utils

B, F, T = 64, 128, 3000
N_CORES = 8
BPC = B // N_CORES  # samples per core
F32 = mybir.dt.float32
BF16 = mybir.dt.bfloat16

_cached = {}


def _build_nc():
    nc = bacc.Bacc("TRN2", target_bir_lowering=False, debug=False)
    x = nc.dram_tensor("x_sh", [BPC, F, T], F32, kind="ExternalInput")
    # row0 = time mask (0/1), row1 = ones
    mtr = nc.dram_tensor("mtr_sh", [BPC, 2, T], BF16, kind="ExternalInput")
    # row0 = ones, row1 = freq mask (0/1)
    mfl = nc.dram_tensor("mfl_sh", [BPC, 2, F], BF16, kind="ExternalInput")
    y = nc.dram_tensor("y_sh", [BPC, F, T], F32, kind="ExternalOutput")

    xa, ta, fa, ya = x.ap(), mtr.ap(), mfl.ap(), y.ap()

    H = T // 2

    with tile.TileContext(nc) as tc:
        with (
            tc.tile_pool(name="xp", bufs=6) as xp,
            tc.tile_pool(name="row", bufs=6) as rowp,
            tc.tile_pool(name="small", bufs=6) as sp,
            tc.tile_pool(name="single", bufs=1) as single,
            tc.tile_pool(name="ps", bufs=2, space="PSUM") as psp,
            tc.tile_pool(name="ps_small", bufs=2, space="PSUM") as psps,
        ):
            ones_row = single.tile([1, F], F32)
            nc.vector.memset(ones_row, 1.0)
            one11 = single.tile([1, 1], F32)
            nc.vector.memset(one11, 1.0)

            for b in range(BPC):
                xt = xp.tile([F, T], F32, tag="xt")
                nc.sync.dma_start(out=xt, in_=xa[b])
                mtb = rowp.tile([2, T], BF16, tag="mtb")
                nc.gpsimd.dma_start(out=mtb, in_=ta[b])
                mfb = sp.tile([2, F], BF16, tag="mfb")
                nc.gpsimd.dma_start(out=mfb, in_=fa[b])

                # combined mask first: PE work depends only on mtb/mfb,
                # so it overlaps the reduce chain below
                ms_halves = []
                for h in range(2):
                    msh = psp.tile([F, H], F32, tag="ms")
                    for c0 in range(0, H, 512):
                        cw = min(512, H - c0)
                        nc.tensor.matmul(
                            msh[:, c0 : c0 + cw],
                            mfb,
                            mtb[:, h * H + c0 : h * H + c0 + cw],
                            start=True,
                            stop=True,
                        )
                    ms_halves.append(msh)

                # per-sample min: free-axis reduce, gather across partitions
                colmin = sp.tile([F, 1], F32, tag="colmin")
                nc.vector.tensor_reduce(
                    out=colmin, in_=xt, axis=mybir.AxisListType.X,
                    op=mybir.AluOpType.min,
                )
                rowmin = sp.tile([1, F], F32, tag="rowmin")
                nc.gpsimd.dma_start(out=rowmin, in_=colmin)
                fill11 = sp.tile([1, 1], F32, tag="fill11")
                nc.vector.tensor_reduce(
                    out=fill11, in_=rowmin, axis=mybir.AxisListType.X,
                    op=mybir.AluOpType.min,
                )
                # fill broadcast [1,1] -> [1,128] (free) -> [128,1] (PE)
                fill_row = sp.tile([1, F], F32, tag="fill_row")
                nc.scalar.mul(fill_row, ones_row, fill11)
                fill128_ps = psps.tile([F, 1], F32, tag="fill128_ps")
                nc.tensor.matmul(fill128_ps, fill_row, one11, start=True, stop=True)
                fill128 = sp.tile([F, 1], F32, tag="fill128")
                nc.scalar.copy(fill128, fill128_ps)

                # nonzero mask => masked cell; overwrite with fill, then
                # store each half as soon as its pred completes
                for h in range(2):
                    nc.vector.copy_predicated(
                        out=xt[:, h * H : (h + 1) * H],
                        mask=ms_halves[h].bitcast(mybir.dt.int32),
                        data=fill128.to_broadcast([F, H]),
                    )
                    nc.scalar.dma_start(
                        out=ya[b][:, h * H : (h + 1) * H],
                        in_=xt[:, h * H : (h + 1) * H],
                    )
    nc.compile()
    return nc


def _host_masks(f0, f_w, t0, t_w):
    nb = f0.shape[0]
    fidx = np.arange(F, dtype=np.int32)
    tidx = np.arange(T, dtype=np.int32)
    fm = (
        (fidx[None, None, :] >= f0[:, :, None])
        & (fidx[None, None, :] < (f0 + f_w)[:, :, None])
    ).any(axis=1)  # [B,F] bool
    tm = (
        (tidx[None, None, :] >= t0[:, :, None])
        & (tidx[None, None, :] < (t0 + t_w)[:, :, None])
    ).any(axis=1)  # [B,T] bool
    mtr = np.ones((nb, 2, T), np.float32)
    mtr[:, 0, :] = tm
    mfl = np.ones((nb, 2, F), np.float32)
    mfl[:, 1, :] = fm
    return mtr.astype(ml_dtypes.bfloat16), mfl.astype(ml_dtypes.bfloat16)


def kernel(x, f0, f_w, t0, t_w, **_):
    x = np.ascontiguousarray(np.asarray(x, dtype=np.float32))
    f0 = np.asarray(f0)
    f_w = np.asarray(f_w)
    t0 = np.asarray(t0)
    t_w = np.asarray(t_w)
    mtr, mfl = _host_masks(f0, f_w, t0, t_w)

    if "nc" not in _cached:
        _cached["nc"] = _build_nc()
    nc = _cached["nc"]

    in_maps = []
    for c in range(N_CORES):
        s = slice(c * BPC, (c + 1) * BPC)
        in_maps.append(
            {
                "x_sh": np.ascontiguousarray(x[s]),
                "mtr_sh": np.ascontiguousarray(mtr[s]),
                "mfl_sh": np.ascontiguousarray(mfl[s]),
            }
        )
    res = bass_utils.run_bass_kernel_spmd(
        nc, in_maps, core_ids=list(range(N_CORES))
    )
    out = np.concatenate([r["y_sh"] for r in res.results], axis=0)
    return out



# revision 3
# speedup vs baseline: 1.0664x; 1.0664x over previous
"""SpecAugment (log-mel masking) Trainium2 kernel.

Full inputs: x [64,128,3000] f32, f0/f_w/t0/t_w [64,2] i32.
out[b,f,t] = fill_b if (f in freq band) or (t in time band) else x[b,f,t],
fill_b = min over x[b].

Strategy: batch-shard B=64 across 8 cores (8 samples/core). The int mask
params are tiny host tensors, so the per-sample 0/1 mask vectors are
computed on host and shipped as bf16 data; the device does only the
memory-bound work (HBM floor: 2 x 12.3MB per core ~= 69us at 358GB/s).

Per core, fully pipelined across the 8 samples:
  - x[b] loaded in 2 column-halves on the sync HWDGE queue (all 8
    samples get distinct SBUF buffers, so every load enqueues up front
    and the load ring never starves)
  - DVE reduce_min per half -> [128,2]; tiny reduce(negate=True) ->
    -min [128,1]; gpsimd partition_all_reduce(max) + negate -> fill on
    all 128 partitions (no SWDGE gather, no PE broadcast round-trip)
  - combined mask = ones(x)mt + mf(x)ones as K=2 bf16 matmuls into
    1-bank PSUM chunks of 500 cols (values {0,1,2}; nonzero == masked);
    all per-sample mask rows come from 2 bulk HWDGE loads at startup
  - DVE copy_predicated per 500-col chunk overwrites masked cells with
    fill (data operand broadcast along free axis)
  - each 1500-col half stored on the scalar HWDGE queue as soon as its
    3 chunks are predicated
"""

import ml_dtypes
import numpy as np

import concourse.bacc as bacc
import concourse.bass as bass
import concourse.bass_isa as bass_isa
import concourse.mybir as mybir
import concourse.tile as tile
import concourse.bass_utils as bass_utils

B, F, T = 64, 128, 3000
N_CORES = 8
BPC = B // N_CORES  # samples per core
H = T // 2          # load/store half width
CW = 500            # mask/pred chunk width (one 2KB PSUM bank of f32)
NCH = T // CW
F32 = mybir.dt.float32
BF16 = mybir.dt.bfloat16

_cached = {}


def _build_nc():
    nc = bacc.Bacc("TRN2", target_bir_lowering=False, debug=False)
    x = nc.dram_tensor("x_sh", [BPC, F, T], F32, kind="ExternalInput")
    # row0 = time mask (0/1), row1 = ones; per-sample slabs of T cols
    mt = nc.dram_tensor("mt_sh", [2, BPC * T], BF16, kind="ExternalInput")
    # row0 = ones, row1 = freq mask (0/1); per-sample slabs of F cols
    mf = nc.dram_tensor("mf_sh", [2, BPC * F], BF16, kind="ExternalInput")
    y = nc.dram_tensor("y_sh", [BPC, F, T], F32, kind="ExternalOutput")

    xa, mta, mfa, ya = x.ap(), mt.ap(), mf.ap(), y.ap()

    with tile.TileContext(nc) as tc:
        with (
            tc.tile_pool(name="xp", bufs=BPC) as xp,
            tc.tile_pool(name="sp", bufs=4) as sp,
            tc.tile_pool(name="mp", bufs=1) as mp,
            tc.tile_pool(name="ps", bufs=8, space="PSUM") as psp,
        ):
            mt_all = mp.tile([2, BPC * T], BF16)
            nc.sync.dma_start(out=mt_all, in_=mta)
            mf_all = mp.tile([2, BPC * F], BF16)
            nc.sync.dma_start(out=mf_all, in_=mfa)

            xts = [None] * BPC
            ncms = [None] * BPC
            fills = [None] * BPC

            def load(b):
                xts[b] = xp.tile([F, T], F32, tag="xt", name=f"xt{b}")
                for h in range(2):
                    nc.sync.dma_start(
                        out=xts[b][:, h * H : (h + 1) * H],
                        in_=xa[b][:, h * H : (h + 1) * H],
                    )

            def minred(b):
                # DVE: free-axis min per half, then -min across the two
                cm = sp.tile([F, 2], F32, tag="cm")
                for h in range(2):
                    nc.vector.tensor_reduce(
                        out=cm[:, h : h + 1],
                        in_=xts[b][:, h * H : (h + 1) * H],
                        axis=mybir.AxisListType.X,
                        op=mybir.AluOpType.min,
                    )
                ncms[b] = sp.tile([F, 1], F32, tag="ncm", name=f"ncm{b}")
                nc.vector.tensor_reduce(
                    out=ncms[b],
                    in_=cm,
                    axis=mybir.AxisListType.X,
                    op=mybir.AluOpType.min,
                    negate=True,
                )

            def fillcalc(b):
                # Pool: max(-colmin) over partitions -> -fill everywhere
                nf = sp.tile([F, 1], F32, tag="nf")
                nc.gpsimd.partition_all_reduce(
                    nf, ncms[b], channels=F, reduce_op=bass_isa.ReduceOp.max
                )
                fills[b] = sp.tile([F, 1], F32, tag="fill", name=f"fill{b}")
                nc.gpsimd.tensor_scalar_mul(fills[b], nf, -1.0)

            def maskpred(b):
                for c in range(NCH):
                    ms = psp.tile([F, CW], F32, tag="ms")
                    nc.tensor.matmul(
                        ms,
                        mf_all[:, b * F : (b + 1) * F],
                        mt_all[:, b * T + c * CW : b * T + (c + 1) * CW],
                        start=True,
                        stop=True,
                    )
                    nc.vector.copy_predicated(
                        out=xts[b][:, c * CW : (c + 1) * CW],
                        mask=ms.bitcast(mybir.dt.int32),
                        data=fills[b].to_broadcast([F, CW]),
                    )
                for h in range(2):
                    nc.scalar.dma_start(
                        out=ya[b][:, h * H : (h + 1) * H],
                        in_=xts[b][:, h * H : (h + 1) * H],
                    )

            load(0)
            load(1)
            minred(0)
            for b in range(BPC):
                if b + 2 < BPC:
                    load(b + 2)
                fillcalc(b)
                maskpred(b)
                if b + 1 < BPC:
                    minred(b + 1)
    nc.compile()
    return nc


def _host_masks(f0, f_w, t0, t_w):
    nb = f0.shape[0]
    fidx = np.arange(F, dtype=np.int32)
    tidx = np.arange(T, dtype=np.int32)
    fm = (
        (fidx[None, None, :] >= f0[:, :, None])
        & (fidx[None, None, :] < (f0 + f_w)[:, :, None])
    ).any(axis=1)  # [B,F] bool
    tm = (
        (tidx[None, None, :] >= t0[:, :, None])
        & (tidx[None, None, :] < (t0 + t_w)[:, :, None])
    ).any(axis=1)  # [B,T] bool
    return fm, tm


def _in_maps(x, f0, f_w, t0, t_w):
    x = np.ascontiguousarray(np.asarray(x, dtype=np.float32))
    fm, tm = _host_masks(
        np.asarray(f0), np.asarray(f_w), np.asarray(t0), np.asarray(t_w)
    )
    maps = []
    for c in range(N_CORES):
        s = slice(c * BPC, (c + 1) * BPC)
        mt2 = np.ones((2, BPC * T), np.float32)
        mt2[0] = tm[s].reshape(-1)
        mf2 = np.ones((2, BPC * F), np.float32)
        mf2[1] = fm[s].reshape(-1)
        maps.append(
            {
                "x_sh": np.ascontiguousarray(x[s]),
                "mt_sh": mt2.astype(ml_dtypes.bfloat16),
                "mf_sh": mf2.astype(ml_dtypes.bfloat16),
            }
        )
    return maps


def kernel(x, f0, f_w, t0, t_w, **_):
    maps = _in_maps(x, f0, f_w, t0, t_w)
    if "nc" not in _cached:
        _cached["nc"] = _build_nc()
    nc = _cached["nc"]
    res = bass_utils.run_bass_kernel_spmd(nc, maps, core_ids=list(range(N_CORES)))
    out = np.concatenate([r["y_sh"] for r in res.results], axis=0)
    return out


# revision 9
# speedup vs baseline: 1.1018x; 1.0332x over previous
"""SpecAugment (log-mel masking) Trainium2 kernel.

Full inputs: x [64,128,3000] f32, f0/f_w/t0/t_w [64,2] i32.
out[b,f,t] = fill_b if (f in freq band) or (t in time band) else x[b,f,t],
fill_b = min over x[b].

Strategy: batch-shard B=64 across 8 cores (8 samples/core). The int mask
params are tiny host tensors, so the per-sample 0/1 mask vectors are
computed on host and shipped as bf16 data; the device does only the
memory-bound work (HBM floor: 2 x 12.3MB per core ~= 69us at 358GB/s).

Per core, fully pipelined across the 8 samples. Engine budget is spread
so the two HWDGE DMA rings stay the pacer:
  - x[b] loaded whole on the sync ring (distinct SBUF buffer per sample
    so every load enqueues up front); mask rows ride the scalar ring so
    the first x load is first in the sync FIFO
  - per-sample min: GpSimd folds x 3000->1500->750 cols with
    tensor_tensor(min); DVE does one cheap 750-col reduce_min
    (negate=True -> -min); GpSimd partition_all_reduce(max) + negate
    puts fill on all 128 partitions
  - combined mask = ones(x)mt + mf(x)ones as K=2 bf16 matmuls (500-col
    chunks) into a [128,1500] PSUM half-tile (values {0,1,2})
  - DVE copy_predicated per 1500-col half overwrites masked cells with
    fill; the half is stored on the scalar ring as soon as it's done
"""

import ml_dtypes
import numpy as np

import concourse.bacc as bacc
import concourse.bass as bass
import concourse.bass_isa as bass_isa
import concourse.mybir as mybir
import concourse.tile as tile
import concourse.bass_utils as bass_utils

B, F, T = 64, 128, 3000
N_CORES = 8
BPC = B // N_CORES  # samples per core
H = T // 2          # pred/store half width
Q = T // 4          # second fold width
CW = 512            # matmul chunk width (= one PSUM bank of f32)
F32 = mybir.dt.float32
BF16 = mybir.dt.bfloat16

_cached = {}


def _build_nc():
    nc = bacc.Bacc("TRN2", target_bir_lowering=False, debug=False)
    x = nc.dram_tensor("x_sh", [BPC, F, T], F32, kind="ExternalInput")
    # row0 = time mask (0/1), row1 = ones; per-sample slabs of T cols
    mt = nc.dram_tensor("mt_sh", [2, BPC * T], BF16, kind="ExternalInput")
    # row0 = ones, row1 = freq mask (0/1); per-sample slabs of F cols
    mf = nc.dram_tensor("mf_sh", [2, BPC * F], BF16, kind="ExternalInput")
    y = nc.dram_tensor("y_sh", [BPC, F, T], F32, kind="ExternalOutput")

    xa, mta, mfa, ya = x.ap(), mt.ap(), mf.ap(), y.ap()

    with tile.TileContext(nc) as tc:
        with (
            tc.tile_pool(name="xp", bufs=BPC) as xp,
            tc.tile_pool(name="sp", bufs=4) as sp,
            tc.tile_pool(name="mp", bufs=1) as mp,
            tc.tile_pool(name="ps", bufs=2, space="PSUM") as psp,
        ):
            # masks ride the (initially idle) scalar/store ring
            mt_all = mp.tile([2, BPC * T], BF16)
            nc.scalar.dma_start(out=mt_all, in_=mta)
            mf_all = mp.tile([2, BPC * F], BF16)
            nc.scalar.dma_start(out=mf_all, in_=mfa)

            xts = [None] * BPC
            ncms = [None] * BPC
            fills = [None] * BPC

            def load(b):
                xts[b] = xp.tile([F, T], F32, tag="xt", name=f"xt{b}")
                if b == 0:
                    # halves so the first reduce overlaps the first load
                    for h in range(2):
                        nc.sync.dma_start(
                            out=xts[b][:, h * H : (h + 1) * H],
                            in_=xa[b][:, h * H : (h + 1) * H],
                        )
                else:
                    nc.sync.dma_start(out=xts[b], in_=xa[b])

            def minred(b):
                # DVE: free-axis min, negated for the Pool max-allreduce
                ncms[b] = sp.tile([F, 1], F32, tag="ncm", name=f"ncm{b}")
                if b == 0:
                    cm = sp.tile([F, 2], F32, tag="cm")
                    for h in range(2):
                        nc.vector.tensor_reduce(
                            out=cm[:, h : h + 1],
                            in_=xts[b][:, h * H : (h + 1) * H],
                            axis=mybir.AxisListType.X,
                            op=mybir.AluOpType.min,
                        )
                    src = cm
                else:
                    src = xts[b]
                nc.vector.tensor_reduce(
                    out=ncms[b],
                    in_=src,
                    axis=mybir.AxisListType.X,
                    op=mybir.AluOpType.min,
                    negate=True,
                )

            def fillcalc(b):
                # Pool: max(-colmin) over partitions -> -fill everywhere
                nf = sp.tile([F, 1], F32, tag="nf")
                nc.gpsimd.partition_all_reduce(
                    nf, ncms[b], channels=F, reduce_op=bass_isa.ReduceOp.max
                )
                fills[b] = sp.tile([F, 1], F32, tag="fill", name=f"fill{b}")
                nc.gpsimd.tensor_scalar_mul(fills[b], nf, -1.0)

            def maskpred(b):
                for h in range(2):
                    ms = psp.tile([F, H], F32, tag="ms")
                    for c0 in range(0, H, CW):
                        cw = min(CW, H - c0)
                        nc.tensor.matmul(
                            ms[:, c0 : c0 + cw],
                            mf_all[:, b * F : (b + 1) * F],
                            mt_all[:, b * T + h * H + c0 : b * T + h * H + c0 + cw],
                            start=True,
                            stop=True,
                        )
                    nc.vector.copy_predicated(
                        out=xts[b][:, h * H : (h + 1) * H],
                        mask=ms.bitcast(mybir.dt.int32),
                        data=fills[b].to_broadcast([F, H]),
                    )
                    nc.scalar.dma_start(
                        out=ya[b][:, h * H : (h + 1) * H],
                        in_=xts[b][:, h * H : (h + 1) * H],
                    )

            load(0)
            load(1)
            minred(0)
            for b in range(BPC):
                if b + 2 < BPC:
                    load(b + 2)
                fillcalc(b)
                maskpred(b)
                if b + 1 < BPC:
                    minred(b + 1)
    nc.compile()
    return nc


def _host_masks(f0, f_w, t0, t_w):
    nb = f0.shape[0]
    fidx = np.arange(F, dtype=np.int32)
    tidx = np.arange(T, dtype=np.int32)
    fm = (
        (fidx[None, None, :] >= f0[:, :, None])
        & (fidx[None, None, :] < (f0 + f_w)[:, :, None])
    ).any(axis=1)  # [B,F] bool
    tm = (
        (tidx[None, None, :] >= t0[:, :, None])
        & (tidx[None, None, :] < (t0 + t_w)[:, :, None])
    ).any(axis=1)  # [B,T] bool
    return fm, tm


def _in_maps(x, f0, f_w, t0, t_w):
    x = np.ascontiguousarray(np.asarray(x, dtype=np.float32))
    fm, tm = _host_masks(
        np.asarray(f0), np.asarray(f_w), np.asarray(t0), np.asarray(t_w)
    )
    maps = []
    for c in range(N_CORES):
        s = slice(c * BPC, (c + 1) * BPC)
        mt2 = np.ones((2, BPC * T), np.float32)
        mt2[0] = tm[s].reshape(-1)
        mf2 = np.ones((2, BPC * F), np.float32)
        mf2[1] = fm[s].reshape(-1)
        maps.append(
            {
                "x_sh": np.ascontiguousarray(x[s]),
                "mt_sh": mt2.astype(ml_dtypes.bfloat16),
                "mf_sh": mf2.astype(ml_dtypes.bfloat16),
            }
        )
    return maps


def kernel(x, f0, f_w, t0, t_w, **_):
    maps = _in_maps(x, f0, f_w, t0, t_w)
    if "nc" not in _cached:
        _cached["nc"] = _build_nc()
    nc = _cached["nc"]
    res = bass_utils.run_bass_kernel_spmd(nc, maps, core_ids=list(range(N_CORES)))
    out = np.concatenate([r["y_sh"] for r in res.results], axis=0)
    return out
